# revision 1
# baseline (speedup 1.0000x reference)
"""2-layer GCN (message passing) on 8 TRN2 NeuronCores via Bass/Tile.

Self-contained: kernel(**inputs) takes the FULL inputs
(x [50000,512] f32, edge_index [2,800000] int, W1 [512,64], b1 [64],
W2 [64,40], b2 [40]) and returns softmax(GCNConv(relu(GCNConv(x)))) as
[50000, 40] f32, computed on 8 NeuronCores.

ReduceScatter formulation (edges sharded by SOURCE core):
  GCN norm factorizes: out = dinv_d * sum_e dinv_s * (x W)[s] + b.
  Nodes are permuted (host-side) so per-dst-tile edge counts balance.
  Per core: g = dinv * (x @ W) for LOCAL nodes only, bf16, rows padded
  to 256B in DRAM. Edges whose src is local are grouped by GLOBAL dst
  tile (392 groups, A-half tiles 0-23 / B-half 24-48 of every core).
  Messages are fetched with dma_gather (Q7 SWDGE path, int16 local
  indices, <=1024 idx per call) and scattered into per-dst-tile psums
  via one-hot bf16 matmuls (sel built by DVE is_equal). Partials
  (pre-scaled by dinv_dst) go to PA/PB buffers; ReduceScatter(add)
  per half sums them across cores, overlapping the B-half compute.
  Layer 1 output: PE-transpose + Act relu(.+b1) into feature-major h.
  Layer 2 repeats with W2, then softmax on local rows.
"""

import math

import numpy as np

# hardcoded problem shapes
N = 50000
IN_C = 512
HID = 64
OC = 40
CORES = 8
P = 128
NPC = N // CORES                 # 6250
NT = math.ceil(NPC / P)          # 49
LAST = NPC - (NT - 1) * P        # 106
KC = IN_C // P                   # 4
TA = 24                          # tiles 0..23  -> half A (full 128 rows)
TB = NT - TA                     # tiles 24..48 -> half B (last tile 106)
GA = CORES * TA                  # 192 A groups
GT = CORES * NT                  # 392 groups total
AROWS = TA * P                   # 3072 rows per core in half A
BROWS = NPC - AROWS              # 3178 rows per core in half B
GCH = 8                          # chunks per dma_gather (1024 idxs)
CHB_MAX = 72                     # chunks per batch (sel/m tile size)
EV_MAX = 24                      # dst tiles per batch (evac tile size)

_CACHE = {}


def _grp_decode(g):
    """group id (A-first order) -> (dst_core, tile_in_core, rows, row0)"""
    if g < GA:
        d, t = divmod(g, TA)
        return d, t, P, g * P
    d, t2 = divmod(g - GA, TB)
    rows = LAST if t2 == TB - 1 else P
    return d, TA + t2, rows, d * BROWS + t2 * P


def _make_plan(kc):
    """Shared (SPMD-uniform) batch plan from per-group chunk counts."""
    choff = np.concatenate([[0], np.cumsum(kc)]).astype(np.int64)
    batches = []
    g = 0
    while g < GT:
        in_a = g < GA
        lim = GA if in_a else GT
        g1 = g
        nch = 0
        while (g1 < lim and g1 - g < EV_MAX and nch + kc[g1] <= CHB_MAX):
            rows = _grp_decode(g1)[2]
            if rows < P and g1 > g:
                break
            nch += kc[g1]
            g1 += 1
            if rows < P:
                break
        batches.append((g, g1, int(choff[g]), int(choff[g1]), in_a))
        g = g1
    return choff, batches


def _build(plan, reps=1, skip=()):
    import concourse.bacc as bacc
    import concourse.tile as tile
    import concourse.mybir as mybir
    from concourse import library_config

    F32 = mybir.dt.float32
    BF16 = mybir.dt.bfloat16
    I16 = mybir.dt.int16
    AF = mybir.ActivationFunctionType
    ALU = mybir.AluOpType

    kc, CH = plan["kc"], plan["CH"]
    choff, batches = plan["choff"], plan["batches"]
    HP = 2 * HID                  # 128-wide (256B) padded message rows

    nc = bacc.Bacc("TRN2", target_bir_lowering=False, debug=False,
                   num_devices=CORES, dynamic_dma_scratch_size=16384)
    xT = nc.dram_tensor("xT", [IN_C, NPC], BF16, kind="ExternalInput")
    w1c = nc.dram_tensor("w1c", [P, KC * HP], BF16, kind="ExternalInput")
    w2p = nc.dram_tensor("w2p", [HID, HP], BF16, kind="ExternalInput")
    b1col = nc.dram_tensor("b1col", [HID, 1], F32, kind="ExternalInput")
    b2rep = nc.dram_tensor("b2rep", [P, HID], F32, kind="ExternalInput")
    dmy = nc.dram_tensor("dmy", [P, NT], F32, kind="ExternalInput")
    dcol = nc.dram_tensor("dcol", [P, GT], F32, kind="ExternalInput")
    idxw = nc.dram_tensor("idxw", [P, CH * 8], I16, kind="ExternalInput")
    dstloc = nc.dram_tensor("dstloc", [P, CH], BF16, kind="ExternalInput")
    iota = nc.dram_tensor("iota", [P, P], BF16, kind="ExternalInput")
    ident = nc.dram_tensor("ident", [P, P], BF16, kind="ExternalInput")
    out = nc.dram_tensor("out", [NPC, OC], F32, kind="ExternalOutput")

    with tile.TileContext(nc) as tc:
        with (
            tc.tile_pool(name="const", bufs=1) as cpool,
            tc.tile_pool(name="dram", bufs=2, space="DRAM") as dram,
            tc.tile_pool(name="hfm", bufs=1) as hpool,
            tc.tile_pool(name="xblk", bufs=8) as xpool,
            tc.tile_pool(name="gath", bufs=2) as gpool,
            tc.tile_pool(name="sel", bufs=2) as spool,
            tc.tile_pool(name="rsl", bufs=2) as rpool,
            tc.tile_pool(name="evac", bufs=4) as epool,
            tc.tile_pool(name="px", bufs=2, space="PSUM") as px,
            tc.tile_pool(name="pnm", bufs=4, space="PSUM") as pnm,
            tc.tile_pool(name="ptr", bufs=2, space="PSUM") as ptr,
        ):
            nc.gpsimd.load_library(library_config.mlp)
            w1_sb = cpool.tile([P, KC, HP], BF16)
            nc.sync.dma_start(out=w1_sb[:], in_=w1c[:].rearrange(
                "p (k h) -> p k h", k=KC))
            w2_sb = cpool.tile([HID, HP], BF16)
            nc.sync.dma_start(out=w2_sb[:], in_=w2p[:])
            b1_sb = cpool.tile([HID, 1], F32)
            nc.sync.dma_start(out=b1_sb[:], in_=b1col[:])
            b2_sb = cpool.tile([P, HID], F32)
            nc.sync.dma_start(out=b2_sb[:], in_=b2rep[:])
            dmy_sb = cpool.tile([P, NT], F32)
            nc.sync.dma_start(out=dmy_sb[:], in_=dmy[:])
            dcol_sb = cpool.tile([P, GT], F32)
            nc.sync.dma_start(out=dcol_sb[:], in_=dcol[:])
            idx_sb = cpool.tile([P, CH * 8], I16)
            nc.sync.dma_start(out=idx_sb[:], in_=idxw[:])
            dloc_sb = cpool.tile([P, CH], BF16)
            nc.sync.dma_start(out=dloc_sb[:], in_=dstloc[:])
            iota_sb = cpool.tile([P, P], BF16)
            nc.sync.dma_start(out=iota_sb[:], in_=iota[:])
            id_sb = cpool.tile([P, P], BF16)
            nc.sync.dma_start(out=id_sb[:], in_=ident[:])

            def dense_layer(w_sb, g_dst):
                """g_dst[t*P + l, :HID] = dinv * (x_or_h @ W); cols HID.. = 0"""
                for t in range(NT):
                    rows = LAST if t == NT - 1 else P
                    psum = px.tile([P, HP], F32, space="PSUM", tag="px")
                    if w_sb is w1_sb:
                        for k in range(KC):
                            xb = xpool.tile([P, P], BF16)
                            nc.sync.dma_start(
                                out=xb[:, :rows],
                                in_=xT[k * P:(k + 1) * P, t * P:t * P + rows])
                            nc.tensor.matmul(
                                out=psum[:rows, :], lhsT=xb[:, :rows],
                                rhs=w_sb[:, k, :], start=(k == 0),
                                stop=(k == KC - 1))
                    else:
                        nc.tensor.matmul(
                            out=psum[:rows, :],
                            lhsT=h_fm[:, t * P:t * P + rows],
                            rhs=w_sb[:], start=True, stop=True)
                    gsb = epool.tile([P, HP], BF16, tag="g")
                    nc.vector.tensor_scalar_mul(
                        out=gsb[:rows, :], in0=psum[:rows, :],
                        scalar1=dmy_sb[:rows, t:t + 1])
                    nc.sync.dma_start(
                        out=g_dst[t * P:t * P + rows, :], in_=gsb[:rows, :])

            def aggregate(g_src, pa, pb):
                """partials over all 392 dst tiles; returns after B evacs.
                Emits RS(A) right after the last A batch via callback."""
                for (g0, g1, ch0, ch1, in_a) in batches:
                    nch_b = ch1 - ch0
                    sel = spool.tile([P, CHB_MAX, P], BF16, tag="sel")
                    if "sel" in skip:
                        nc.vector.memset(sel[:, 0:1, :], 0.0)
                    else:
                        nc.vector.tensor_tensor(
                            out=sel[:, :nch_b, :],
                            in0=dloc_sb[:, ch0:ch1].unsqueeze(2).to_broadcast(
                                [P, nch_b, P]),
                            in1=iota_sb[:].unsqueeze(1).to_broadcast(
                                [P, nch_b, P]),
                            op=ALU.is_equal)
                    m = gpool.tile([P, CHB_MAX, HP], BF16, tag="m")
                    if "gather" in skip:
                        nc.vector.memset(m[:, 0:1, :], 0.0)
                    else:
                        r0 = ch0
                        while r0 < ch1:
                            r1 = min(r0 + GCH, ch1)
                            nidx = (r1 - r0) * P
                            nc.gpsimd.dma_gather(
                                m[:, r0 - ch0:r1 - ch0, :], g_src[:],
                                idx_sb[:, r0 * 8:r1 * 8], nidx, nidx, HP,
                                single_packet=False)
                            r0 = r1
                    ev = epool.tile([P, EV_MAX, HID], BF16, tag="ev")
                    for gi in range(g0, g1):
                        psum = pnm.tile([P, HID], F32, space="PSUM")
                        for j in range(int(kc[gi])):
                            ch = int(choff[gi]) + j
                            nc.tensor.matmul(
                                out=psum[:], lhsT=sel[:, ch - ch0, :],
                                rhs=m[:, ch - ch0, :HID],
                                start=(j == 0), stop=(j == int(kc[gi]) - 1))
                        nc.vector.tensor_scalar_mul(
                            out=ev[:, gi - g0, :], in0=psum[:],
                            scalar1=dcol_sb[:, gi:gi + 1])
                    d0, t0, rows_l, row0 = _grp_decode(g0)
                    ngr = g1 - g0
                    nrow_full = (ngr - 1) * P + _grp_decode(g1 - 1)[2]
                    dst = pa if in_a else pb
                    nc.sync.dma_start(
                        out=dst[row0:row0 + nrow_full, :].rearrange(
                            "(g p) h -> p g h", p=P) if nrow_full == ngr * P
                        else dst[row0:row0 + nrow_full, :].unsqueeze(1),
                        in_=ev[:, :ngr, :] if nrow_full == ngr * P
                        else ev[:nrow_full, 0, :].unsqueeze(1))
                    if g1 == GA:
                        yield "A"
                yield "B"

            def rs_half(p_in, rows_per_core, tag):
                r = dram.tile([rows_per_core, HID], BF16, tag=tag)
                if "rs" not in skip:
                    nc.gpsimd.collective_compute(
                        "ReduceScatter", ALU.add,
                        ins=[p_in[:].opt()], outs=[r[:].opt()],
                        replica_groups=[list(range(CORES))])
                return r

            def load_half(rs_a, rs_b, tag):
                """rs halves -> SBUF [P, NT, HID] bf16 (tile t at [:, t, :])"""
                lt = rpool.tile([P, NT, HID], BF16, tag=tag)
                nc.sync.dma_start(
                    out=lt[:, :TA, :],
                    in_=rs_a[:].rearrange("(t p) h -> p t h", p=P))
                nc.sync.dma_start(
                    out=lt[:, TA:NT - 1, :],
                    in_=rs_b[:(TB - 1) * P, :].rearrange(
                        "(t p) h -> p t h", p=P))
                nc.vector.memset(lt[:, NT - 1, :], 0.0)
                nc.sync.dma_start(
                    out=lt[:LAST, NT - 1, :],
                    in_=rs_b[(TB - 1) * P:, :])
                return lt

            def load_self(g_loc, tag):
                """own g rows node-major -> SBUF [P, NT, HID] bf16"""
                st = rpool.tile([P, NT, HID], BF16, tag=tag)
                nc.sync.dma_start(
                    out=st[:, :NT - 1, :],
                    in_=g_loc[:(NT - 1) * P, :HID].rearrange(
                        "(t p) h -> p t h", p=P))
                nc.vector.memset(st[:, NT - 1, :], 0.0)
                nc.sync.dma_start(
                    out=st[:LAST, NT - 1, :],
                    in_=g_loc[(NT - 1) * P:, :HID])
                return st

            for _rep in range(reps):
                g1_loc = dram.tile([NPC, HP], BF16, tag="g1")
                g2_loc = dram.tile([NPC, HP], BF16, tag="g2")
                pa1 = dram.tile([CORES * AROWS, HID], BF16, tag="pa1")
                pb1 = dram.tile([CORES * BROWS, HID], BF16, tag="pb1")
                pa2 = dram.tile([CORES * AROWS, HID], BF16, tag="pa2")
                pb2 = dram.tile([CORES * BROWS, HID], BF16, tag="pb2")

                # ---- layer 1 ----
                dense_layer(w1_sb, g1_loc)
                rs = {}
                for half in aggregate(g1_loc, pa1, pb1):
                    if half == "A":
                        rs["A"] = rs_half(pa1, AROWS, "rsa1")
                    else:
                        rs["B"] = rs_half(pb1, BROWS, "rsb1")
                h_load = load_half(rs["A"], rs["B"], "h")
                s1 = load_self(g1_loc, "s1")
                h_fm = hpool.tile([HID, NT * P], BF16)
                for t in range(NT):
                    tmp = epool.tile([P, HID], F32, tag="st")
                    nc.vector.tensor_scalar_mul(
                        out=tmp[:], in0=s1[:, t, :],
                        scalar1=dmy_sb[:, t:t + 1])
                    zsum = epool.tile([P, HID], BF16, tag="zs")
                    nc.vector.tensor_tensor(
                        out=zsum[:], in0=tmp[:], in1=h_load[:, t, :],
                        op=ALU.add)
                    pt = ptr.tile([HID, P], BF16, space="PSUM")
                    nc.tensor.transpose(
                        out=pt[:], in_=zsum[:], identity=id_sb[:])
                    nc.scalar.activation(
                        out=h_fm[:, t * P:(t + 1) * P], in_=pt[:],
                        func=AF.Relu, bias=b1_sb[:, 0:1], scale=1.0)

                # ---- layer 2 ----
                dense_layer(w2_sb, g2_loc)
                rs2 = {}
                for half in aggregate(g2_loc, pa2, pb2):
                    if half == "A":
                        rs2["A"] = rs_half(pa2, AROWS, "rsa2")
                    else:
                        rs2["B"] = rs_half(pb2, BROWS, "rsb2")
                z_load = load_half(rs2["A"], rs2["B"], "z")
                s2 = load_self(g2_loc, "s2")
                for t in range(NT):
                    rows = LAST if t == NT - 1 else P
                    tmp = epool.tile([P, HID], F32, tag="st")
                    nc.vector.tensor_scalar_mul(
                        out=tmp[:], in0=s2[:, t, :],
                        scalar1=dmy_sb[:, t:t + 1])
                    nc.vector.tensor_tensor(
                        out=tmp[:], in0=tmp[:], in1=z_load[:, t, :],
                        op=ALU.add)
                    z = epool.tile([P, HID], F32, tag="z")
                    nc.vector.tensor_tensor(
                        out=z[:], in0=tmp[:], in1=b2_sb[:],
                        op=ALU.add)
                    nmax = epool.tile([P, 1], F32, tag="nmax")
                    nc.vector.tensor_reduce(
                        out=nmax[:], in_=z[:, :OC], axis=mybir.AxisListType.X,
                        op=ALU.max, negate=True)
                    e = epool.tile([P, OC], F32, tag="e")
                    nc.scalar.activation(
                        out=e[:], in_=z[:, :OC], func=AF.Exp,
                        bias=nmax[:, 0:1], scale=1.0)
                    ssum = epool.tile([P, 1], F32, tag="ssum")
                    nc.vector.tensor_reduce(
                        out=ssum[:], in_=e[:], axis=mybir.AxisListType.X,
                        op=ALU.add)
                    rec = epool.tile([P, 1], F32, tag="rec")
                    nc.vector.reciprocal(out=rec[:], in_=ssum[:])
                    o = epool.tile([P, OC], F32, tag="o")
                    nc.vector.tensor_scalar_mul(
                        out=o[:], in0=e[:], scalar1=rec[:, 0:1])
                    nc.sync.dma_start(
                        out=out[t * P:t * P + rows, :], in_=o[:rows, :])

    nc.compile()
    return nc


def _balance_nodes(deg):
    """Assign nodes to 392 global tiles (49/core, last tile 106 slots)
    so per-tile incoming-edge counts are near-equal. Returns newid[v]."""
    import heapq
    cap = np.full(GT, P, dtype=np.int64)
    cap[NT - 1::NT] = LAST
    order = np.argsort(-deg, kind="stable")
    heap = [(0.0, float(g)) for g in range(GT)]
    heapq.heapify(heap)
    fill = np.zeros(GT, dtype=np.int64)
    gtile = np.empty(N, dtype=np.int64)
    local = np.empty(N, dtype=np.int64)
    for v in order:
        while True:
            load, gf = heapq.heappop(heap)
            g = int(gf)
            if fill[g] < cap[g]:
                break
        gtile[v] = g
        local[v] = fill[g]
        fill[g] += 1
        if fill[g] < cap[g]:
            heapq.heappush(heap, (load + float(deg[v]), gf))
    core = gtile // NT
    t_in = gtile % NT
    newid = core * NPC + t_in * P + local
    return newid


def _preprocess(x, edge_index, W1, b1, W2, b2):
    import ml_dtypes
    bf16 = ml_dtypes.bfloat16
    HP = 2 * HID

    src = edge_index[0]
    dst = edge_index[1]
    deg_real = np.bincount(dst, minlength=N).astype(np.float32)
    deg = deg_real + 1.0                      # self-loops (PyG default)
    dinv = (deg ** -0.5).astype(np.float32)

    newid = _balance_nodes(deg_real)
    nodeat = np.empty(N, dtype=np.int64)      # newid -> original node
    nodeat[newid] = np.arange(N)
    dinv_n = dinv[nodeat]                     # dinv indexed by newid

    src_n = newid[src]
    dst_n = newid[dst]
    src_core = src_n // NPC
    w_d = dst_n % NPC
    dst_core = dst_n // NPC
    t_in = w_d // P
    loc = (w_d % P).astype(np.float32)
    isB = t_in >= TA
    okey = np.where(isB, GA + dst_core * TB + (t_in - TA),
                    dst_core * TA + t_in)

    cell = src_core * GT + okey
    counts = np.bincount(cell, minlength=CORES * GT).reshape(CORES, GT)
    kc = np.maximum(1, -(-counts.max(axis=0) // P)).astype(np.int64)
    choff = np.concatenate([[0], np.cumsum(kc)]).astype(np.int64)
    CH = int(choff[-1])

    order = np.argsort(cell, kind="stable")
    cs = cell[order]
    cell_counts = np.bincount(cell, minlength=CORES * GT)
    starts = np.zeros(CORES * GT, dtype=np.int64)
    np.cumsum(cell_counts[:-1], out=starts[1:])
    rank = np.arange(len(cs)) - starts[cs]
    core_o = cs // GT
    okey_o = cs % GT
    slot = core_o * (CH * P) + choff[okey_o] * P + rank

    src_flat = np.zeros(CORES * CH * P, dtype=np.int16)
    loc_flat = np.full(CORES * CH * P, -1.0, dtype=np.float32)
    src_flat[slot] = (src_n[order] % NPC).astype(np.int16)
    loc_flat[slot] = loc[order]

    iota = np.broadcast_to(
        np.arange(P, dtype=np.float32)[None, :], (P, P)).astype(bf16).copy()
    ident = np.eye(P, dtype=np.float32).astype(bf16)
    w1pad = np.zeros((IN_C, HP), np.float32)
    w1pad[:, :HID] = W1
    w1c = np.ascontiguousarray(
        w1pad.reshape(KC, P, HP).transpose(1, 0, 2).reshape(
            P, KC * HP).astype(bf16))
    w2pad = np.zeros((HID, HP), bf16)
    w2pad[:, :OC] = W2.astype(bf16)
    b2p = np.zeros(HID, np.float32)
    b2p[:OC] = b2.astype(np.float32)
    b2r = np.broadcast_to(b2p[None, :], (P, HID)).copy()

    # dcol[p, g] = dinv of dst node (tile g, local p), 0 on pad rows
    dcol = np.zeros((P, GT), np.float32)
    for g in range(GT):
        d, t, rows, _ = _grp_decode(g)
        base = d * NPC + t * P
        dcol[:rows, g] = dinv_n[base:base + rows]

    x_bf = x.astype(bf16)
    in_maps = []
    for c in range(CORES):
        c0 = c * NPC
        dv_pad = np.zeros(NT * P, dtype=np.float32)
        dv_pad[:NPC] = dinv_n[c0:c0 + NPC]
        sl = slice(c * CH * P, (c + 1) * CH * P)
        arr = src_flat[sl]
        wrapped = np.tile(
            np.ascontiguousarray(arr.reshape(CH * 8, 16).T), (8, 1))
        in_maps.append({
            "xT": np.ascontiguousarray(x_bf[nodeat[c0:c0 + NPC]].T),
            "w1c": w1c,
            "w2p": w2pad,
            "b1col": b1.reshape(HID, 1).astype(np.float32),
            "b2rep": b2r,
            "dmy": np.ascontiguousarray(dv_pad.reshape(NT, P).T),
            "dcol": dcol,
            "idxw": np.ascontiguousarray(wrapped),
            "dstloc": np.ascontiguousarray(
                loc_flat[sl].reshape(CH, P).T.astype(bf16)),
            "iota": iota,
            "ident": ident,
        })
    choff_p, batches = _make_plan(kc)
    assert np.array_equal(choff_p, choff)
    plan = {"kc": kc, "CH": CH, "choff": choff, "batches": batches}
    return in_maps, plan, nodeat


class _Runner:
    """SPMD launch via the axon/PJRT path (shard_map over 8 NeuronCores)."""

    def __init__(self, nc):
        import jax
        from jax.sharding import Mesh, PartitionSpec
        from jax.experimental.shard_map import shard_map
        import concourse.mybir as mybir
        from concourse import bass2jax

        bass2jax.install_neuronx_cc_hook()
        self.jax = jax
        partition_name = (
            nc.partition_id_tensor.name if nc.partition_id_tensor else None)
        in_names, out_names, out_avals, zero_outs = [], [], [], []
        for alloc in nc.m.functions[0].allocations:
            if not isinstance(alloc, mybir.MemoryLocationSet):
                continue
            name = alloc.memorylocations[0].name
            if alloc.kind == "ExternalInput":
                if name != partition_name:
                    in_names.append(name)
            elif alloc.kind == "ExternalOutput":
                shape = tuple(alloc.tensor_shape)
                dtype = mybir.dt.np(alloc.dtype)
                out_names.append(name)
                out_avals.append(jax.core.ShapedArray(shape, dtype))
                zero_outs.append(np.zeros(shape, dtype))
        self.in_names, self.out_names = in_names, out_names
        self.out_avals, self.zero_outs = out_avals, zero_outs
        n_params, n_outs = len(in_names), len(out_names)
        all_in = in_names + out_names
        if partition_name is not None:
            all_in.append(partition_name)

        def _body(*args):
            operands = list(args)
            if partition_name is not None:
                operands.append(bass2jax.partition_id_tensor())
            outs = bass2jax._bass_exec_p.bind(
                *operands,
                out_avals=tuple(out_avals),
                in_names=tuple(all_in),
                out_names=tuple(out_names),
                lowering_input_output_aliases=(),
                sim_require_finite=True,
                sim_require_nnan=True,
                nc=nc,
            )
            return tuple(outs)

        devices = jax.devices()[:CORES]
        mesh = Mesh(np.asarray(devices), ("core",))
        self.fn = jax.jit(
            shard_map(
                _body, mesh=mesh,
                in_specs=(PartitionSpec("core"),) * (n_params + n_outs),
                out_specs=(PartitionSpec("core"),) * n_outs,
                check_rep=False),
            donate_argnums=tuple(range(n_params, n_params + n_outs)),
            keep_unused=True)

    def run(self, in_maps):
        jax = self.jax
        concat_in = [
            np.concatenate([np.ascontiguousarray(in_maps[c][k])
                            for c in range(CORES)], axis=0)
            for k in self.in_names
        ]
        zeros = [
            np.zeros((CORES * z.shape[0], *z.shape[1:]), z.dtype)
            for z in self.zero_outs
        ]
        outs = self.fn(*concat_in, *zeros)
        jax.block_until_ready(outs)
        res = np.asarray(outs[0]).reshape(CORES, *self.out_avals[0].shape)
        return res


def kernel(x, edge_index, W1, b1, W2, b2):
    x = np.asarray(x, dtype=np.float32)
    edge_index = np.asarray(edge_index, dtype=np.int64)
    W1 = np.asarray(W1, dtype=np.float32)
    b1 = np.asarray(b1, dtype=np.float32)
    W2 = np.asarray(W2, dtype=np.float32)
    b2 = np.asarray(b2, dtype=np.float32)

    in_maps, plan, nodeat = _preprocess(x, edge_index, W1, b1, W2, b2)
    key = ("gcn3", plan["CH"], plan["kc"].tobytes())
    if key not in _CACHE:
        nc = _build(plan)
        _CACHE[key] = _Runner(nc)
    res = _CACHE[key].run(in_maps)
    full = np.concatenate([res[c] for c in range(CORES)], axis=0)
    outv = np.empty_like(full)
    outv[nodeat] = full
    return outv


if __name__ == "__main__":
    import reference  # only when run manually next to reference.py
    inputs = reference.setup_inputs()
    outv = kernel(**{k: np.asarray(v) for k, v in inputs.items()})
    print("out", outv.shape, outv.dtype)

